# revision 28
# baseline (speedup 1.0000x reference)
"""Trainium2 Bass kernel for nn_AttnTopDown (sparse local attention, 2 layers).

Sharding: 8 cores = 4 batches x 2 spatial halves (top-grid row halves, 1 halo
row).  Each core computes both attn layers on its shard; p5 passes through on
host.

Per-core algorithm (channel-major: SBUF partition p holds channels p and
p+128):
  q = Wq@bot, v = Wv@bot, k = Wk@top       (PE)
  prod_k2 = shift(q, k2) * k               (DVE, 9 window offsets)
  logits72[(k2,h), pos] = masked-matmul(prod)   (PE, M=72 accumulate)
  exp -> den (PE mask reduce) -> recip -> r72 (PE expand) -> wnorm
  S_exp[c, botpos] = sum_k2 fold-matmuls of wnorm with parity-shifted views
  out = v . S_exp ; y = Ww1 @ relu(Ww0 @ out)    (DVE + PE)

Blocks of RT top rows are software-pipelined: softmax(i-1), convs(i),
fold/y-convs(i-1) so every engine keeps dense independent work.
"""

import math
import numpy as np

C = 256
NH = 8
HD = 32  # head dim
K2 = 9

# layer geometry
L0 = dict(T=33, W=64, X=128, U=67, RT=8, NBLK=4)
L1 = dict(T=17, W=32, X=64, U=35, RT=8, NBLK=2)

_PROGRAM = None
_last_in_maps = None


class _LayerRun:
    def __init__(self, nc, tc, pools, L, io, consts):
        import concourse.mybir as mybir
        self.nc = nc
        self.mybir = mybir
        self.L, self.io, self.consts = L, io, consts
        (self.wpool, self.kpool, self.inpool, self.qpool, self.vpool,
         self.prodpool, self.attpool, self.outpool, self.ypool,
         self.mmps) = pools
        self.T, self.W, self.X, self.U = L["T"], L["W"], L["X"], L["U"]
        self.RT, self.NBLK = L["RT"], L["NBLK"]
        self.BR = 2 * self.RT + 3
        self.PW = self.X + 2
        self.TWP = self.W + 1
        self.qr = 512 // self.X
        self.RC = 512 // self.W

    def cview(self, d):
        return d[:].rearrange("(ki p) r x -> p ki r x", p=128)

    def setup(self):
        nc, dt = self.nc, self.mybir.dt
        self.wt = {}
        for nm, key in (("k", "wkT"), ("q", "wqT"), ("v", "wvT"),
                        ("o0", "wo0T"), ("o1", "wo1T")):
            d = self.io[key]
            t = self.wpool.tile([128, 2, 2, 128], d.dtype, tag=f"w{nm}", name=f"w{nm}")
            nc.sync.dma_start(out=t[:], in_=d[:].rearrange(
                "(ki p) (mi m) -> p ki mi m", p=128, m=128))
            self.wt[nm] = t
            if nm == "k":
                T, W = self.T, self.W
                ktile = self.kpool.tile([128, 2, T, W], self.io["top"].dtype,
                                        tag="ktile", name="ktile")
                nc.sync.dma_start(out=ktile[:], in_=self.cview(self.io["top"]))

        T, W = self.T, self.W
        self.k_sb = self.kpool.tile([128, 2, T, W], dt.bfloat16, tag="k_sb", name="k_sb")
        kr = 512 // W
        r0 = 0
        while r0 < T:
            nr = min(kr, T - r0)
            for mi in range(2):
                ps = self.mmps.tile([128, 512], dt.float32, tag="mmps", name="mmps")
                for ki in range(2):
                    nc.tensor.matmul(
                        ps[:, : nr * W],
                        self.wt["k"][:, ki, mi, :],
                        ktile[:, ki, r0 : r0 + nr, :],
                        start=(ki == 0), stop=(ki == 1))
                nc.scalar.copy(out=self.k_sb[:, mi, r0 : r0 + nr, :],
                               in_=ps[:, : nr * W])
            r0 += nr

    def front_a(self, ib):
        nc, dt = self.nc, self.mybir.dt
        RT, W, X, BR, PW, qr = self.RT, self.W, self.X, self.BR, self.PW, self.qr
        ta = ib * RT
        bt = self.inpool.tile([128, 2, BR, X], self.io["bot"].dtype, tag="bt", name="bt")
        nc.sync.dma_start(out=bt[:],
                          in_=self.cview(self.io["bot"])[:, :, 2 * ta : 2 * ta + BR, :])

        qpad = self.qpool.tile([128, 2, BR, PW], dt.bfloat16, tag="qpad", name="qpad")
        nc.vector.memset(qpad[:, :, :, 0:1], 0.0)
        nc.vector.memset(qpad[:, :, :, X + 1 : X + 2], 0.0)
        st = dict(ib=ib, ta=ta, bt=bt, qpad=qpad)
        self._qconv(st, 0)
        self._qconv(st, self.qr)
        return st

    def _qconv(self, st, r0):
        nc, dt = self.nc, self.mybir.dt
        X, BR, qr = self.X, self.BR, self.qr
        nr = min(qr, BR - r0)
        for mi in range(2):
            ps = self.mmps.tile([128, 512], dt.float32, tag="mmps", name="mmps")
            for ki in range(2):
                nc.tensor.matmul(
                    ps[:, : nr * X],
                    self.wt["q"][:, ki, mi, :],
                    st["bt"][:, ki, r0 : r0 + nr, :],
                    start=(ki == 0), stop=(ki == 1))
            nc.scalar.copy(
                out=st["qpad"][:, mi, r0 : r0 + nr, 1 : X + 1], in_=ps[:, : nr * X])

    def front_b(self, st):
        nc, dt = self.nc, self.mybir.dt
        RT, W, X, BR, qr = self.RT, self.W, self.X, self.BR, self.qr
        r0 = 2 * qr
        while r0 < BR:
            self._qconv(st, r0)
            r0 += min(qr, BR - r0)

        vtile = self.vpool.tile([128, 2, 2 * RT, X], dt.bfloat16, tag="vtile", name="vtile")
        for c in range(2 * RT // qr):
            r0 = c * qr
            for mi in range(2):
                ps = self.mmps.tile([128, 512], dt.float32, tag="mmps", name="mmps")
                for ki in range(2):
                    nc.tensor.matmul(
                        ps[:, : qr * X],
                        self.wt["v"][:, ki, mi, :],
                        st["bt"][:, ki, 1 + r0 : 1 + r0 + qr, :],
                        start=(ki == 0), stop=(ki == 1))
                nc.scalar.copy(out=vtile[:, mi, r0 : r0 + qr, :], in_=ps[:, : qr * X])
        st["vtile"] = vtile

    def front_c(self, st):
        nc, dt = self.nc, self.mybir.dt
        RT, W = self.RT, self.W
        ta, qpad = st["ta"], st["qpad"]
        prods = []
        for k2 in range(K2):
            di, dj = k2 // 3, k2 % 3
            pr = self.prodpool.tile([128, 2, RT + 1, W], dt.bfloat16,
                                    tag="prod", name="prod")
            qv = qpad[:, :, di : di + 2 * RT + 1 : 2, dj : dj + 2 * W : 2]
            kv = self.k_sb[:, :, ta : ta + RT + 1, :]
            nc.vector.tensor_mul(pr[:], qv, kv)
            prods.append(pr)

        hr_sb = self.consts["hr"]
        TWB = (RT + 1) * W
        lg = self.mmps.tile([72, TWB], dt.float32, tag="lgw", name="lgw", bufs=1)
        rr = 0
        while rr < RT + 1:
            nr = min(self.RC, RT + 1 - rr)
            nn = nr * W
            for k2 in range(K2):
                for ki in range(2):
                    nc.tensor.matmul(
                        lg[:, rr * W : rr * W + nn],
                        hr_sb[:, ki, k2, :],
                        prods[k2][:, ki, rr : rr + nr, :],
                        start=(k2 == 0 and ki == 0),
                        stop=(k2 == K2 - 1 and ki == 1))
            rr += nr
        st["lg"] = lg

    def back1(self, st):
        nc, dt = self.nc, self.mybir.dt
        AF = self.mybir.ActivationFunctionType
        RT, W, TWP = self.RT, self.W, self.TWP
        den_sb, exp_sb = self.consts["den"], self.consts["exp"]
        TWB = (RT + 1) * W
        lg = st["lg"]
        w72 = self.attpool.tile([72, RT + 1, TWP], dt.bfloat16, tag="w72", name="w72")
        nc.vector.memset(w72[:, :, W : W + 1], 0.0)
        e72 = self.attpool.tile([72, TWB], dt.float32r, tag="e72", name="e72")
        nc.scalar.activation(out=e72[:], in_=lg[:], func=AF.Exp)
        rec = self.attpool.tile([8, TWB], dt.float32r, tag="rec", name="rec")
        for n0 in range(0, TWB, 512):
            nn = min(512, TWB - n0)
            den = self.mmps.tile([8, 512], dt.float32, tag="mmps", name="den")
            nc.tensor.matmul(den[:, :nn], den_sb[:], e72[:, n0 : n0 + nn],
                             start=True, stop=True)
            nc.vector.reciprocal(out=rec[:, n0 : n0 + nn], in_=den[:, :nn])
        r72 = self.mmps.tile([72, TWB], dt.float32, tag="lgw", name="r72", bufs=1)
        for n0 in range(0, TWB, 512):
            nn = min(512, TWB - n0)
            nc.tensor.matmul(r72[:, n0 : n0 + nn], exp_sb[:], rec[:, n0 : n0 + nn],
                             start=True, stop=True)
        nc.vector.tensor_mul(
            w72[:, :, 0:W],
            e72[:].rearrange("p (a b) -> p a b", b=W),
            r72[:].rearrange("p (a b) -> p a b", b=W))
        if st["ib"] == self.NBLK - 1:
            nc.vector.tensor_scalar_mul(w72[:, RT, 0:W], w72[:, RT, 0:W],
                                        self.consts["halo"][:])
        st["w72"] = w72

    def back2a(self, st):
        nc, dt = self.nc, self.mybir.dt
        RT, W, X = self.RT, self.W, self.X
        fold_sb = self.consts["fold"]
        vtile, w72 = st["vtile"], st["w72"]
        outb = self.outpool.tile([128, 2, 2 * RT, X], dt.float32r, tag="outb", name="outb")
        for ki in range(2):
            for pu in range(2):
                for pv in range(2):
                    terms = [(di, dj)
                             for di in ((1,) if pu == 0 else (0, 2))
                             for dj in ((1,) if pv == 0 else (0, 2))]
                    nn = RT * (X // 2)
                    se = self.mmps.tile([128, 512], dt.float32, tag="mmps", name="se")
                    for i, (di, dj) in enumerate(terms):
                        dr, dc = (1 if di == 0 else 0), (1 if dj == 0 else 0)
                        k2 = di * 3 + dj
                        wv_ = w72[:, dr : dr + RT, dc : dc + W]
                        nc.tensor.matmul(
                            se[:, :nn], fold_sb[:, ki, k2, :], wv_,
                            start=(i == 0), stop=(i == len(terms) - 1))
                    nc.vector.tensor_mul(
                        outb[:, ki, pu :: 2, pv :: 2],
                        vtile[:, ki, pu :: 2, pv :: 2],
                        se[:, :nn].rearrange("p (a b) -> p a b", b=W))
        st["outb"] = outb

    def back2b(self, st):
        nc, dt = self.nc, self.mybir.dt
        AF = self.mybir.ActivationFunctionType
        RT, W, X, qr = self.RT, self.W, self.X, self.qr
        ta, outb = st["ta"], st["outb"]
        for c in range(2 * RT // qr):
            r0 = c * qr
            y0r = self.ypool.tile([128, 2, qr * X], dt.float32r, tag="y0r", name="y0r")
            for mi in range(2):
                ps = self.mmps.tile([128, 512], dt.float32, tag="mmps", name="mmps")
                for ki in range(2):
                    nc.tensor.matmul(
                        ps[:, : qr * X],
                        self.wt["o0"][:, ki, mi, :],
                        outb[:, ki, r0 : r0 + qr, :],
                        start=(ki == 0), stop=(ki == 1))
                nc.scalar.activation(out=y0r[:, mi, :], in_=ps[:, : qr * X],
                                     func=AF.Relu)
            yout = self.ypool.tile([128, 2, qr, X], dt.float32, tag="yout", name="yout")
            for mi in range(2):
                ps = self.mmps.tile([128, 512], dt.float32, tag="mmps", name="mmps")
                for ki in range(2):
                    nc.tensor.matmul(
                        ps[:, : qr * X],
                        self.wt["o1"][:, ki, mi, :],
                        y0r[:, ki, :],
                        start=(ki == 0), stop=(ki == 1))
                if mi == 0:
                    nc.vector.tensor_copy(yout[:, mi, :, :], ps[:, : qr * X])
                else:
                    nc.scalar.copy(out=yout[:, mi, :, :], in_=ps[:, : qr * X])
            nc.sync.dma_start(
                out=self.cview(self.io["out"])[:, :, 2 * ta + r0 : 2 * ta + r0 + qr, :],
                in_=yout[:])


def build_program():
    import concourse.bacc as bacc
    import concourse.mybir as mybir
    import concourse.tile as tile
    from contextlib import ExitStack

    dt = mybir.dt
    nc = bacc.Bacc(None, target_bir_lowering=False)

    def make_io(sfx, L):
        return dict(
            bot=nc.dram_tensor(f"bot{sfx}", [C, L["U"], L["X"]], dt.bfloat16,
                               kind="ExternalInput"),
            top=nc.dram_tensor(f"top{sfx}", [C, L["T"], L["W"]], dt.bfloat16,
                               kind="ExternalInput"),
            out=nc.dram_tensor(f"out{sfx}", [C, 2 * L["RT"] * L["NBLK"], L["X"]],
                               dt.float32, kind="ExternalOutput"),
            wqT=nc.dram_tensor(f"wqT{sfx}", [C, C], dt.bfloat16, kind="ExternalInput"),
            wkT=nc.dram_tensor(f"wkT{sfx}", [C, C], dt.bfloat16, kind="ExternalInput"),
            wvT=nc.dram_tensor(f"wvT{sfx}", [C, C], dt.bfloat16, kind="ExternalInput"),
            wo0T=nc.dram_tensor(f"wo0T{sfx}", [C, C], dt.float32r, kind="ExternalInput"),
            wo1T=nc.dram_tensor(f"wo1T{sfx}", [C, C], dt.float32r, kind="ExternalInput"),
        )

    io0, io1 = make_io(0, L0), make_io(1, L1)
    hr_d = nc.dram_tensor("hr_mask", [128, 2, K2, 72], dt.bfloat16, kind="ExternalInput")
    fold_d = nc.dram_tensor("fold_mask", [72, 2, K2, 128], dt.bfloat16, kind="ExternalInput")
    den_d = nc.dram_tensor("den_mask", [72, 8], dt.float32r, kind="ExternalInput")
    exp_d = nc.dram_tensor("exp_mask", [8, 72], dt.float32r, kind="ExternalInput")
    halo_d = nc.dram_tensor("halo", [72, 1], dt.float32, kind="ExternalInput")

    with tile.TileContext(nc) as tc:
        octx0 = ExitStack()
        octx0.enter_context(nc.allow_low_precision(reason="f32r rounding is intentional"))
        with octx0, ExitStack() as octx:
            cpool = octx.enter_context(tc.tile_pool(name="consts", bufs=1))
            consts = dict(
                hr=cpool.tile([128, 2, K2, 72], dt.bfloat16, tag="hr", name="hr"),
                fold=cpool.tile([72, 2, K2, 128], dt.bfloat16, tag="fold", name="fold"),
                den=cpool.tile([72, 8], dt.float32r, tag="den", name="den"),
                exp=cpool.tile([8, 72], dt.float32r, tag="exp", name="exp"),
                halo=cpool.tile([72, 1], dt.float32, tag="halo", name="halo"),
            )
            pools = (
                octx.enter_context(tc.tile_pool(name="wpool", bufs=1)),
                octx.enter_context(tc.tile_pool(name="kpool", bufs=1)),
                octx.enter_context(tc.tile_pool(name="inpool", bufs=2)),
                octx.enter_context(tc.tile_pool(name="qpool", bufs=2)),
                octx.enter_context(tc.tile_pool(name="vpool", bufs=2)),
                octx.enter_context(tc.tile_pool(name="prodpool", bufs=12)),
                octx.enter_context(tc.tile_pool(name="attpool", bufs=4)),
                octx.enter_context(tc.tile_pool(name="outpool", bufs=2)),
                octx.enter_context(tc.tile_pool(name="ypool", bufs=3)),
                octx.enter_context(tc.tile_pool(name="mmps", bufs=6, space="PSUM")),
            )
            # all blocks of both layers in one software pipeline
            l0 = _LayerRun(nc, tc, pools, L0, io0, consts)
            l1 = _LayerRun(nc, tc, pools, L1, io1, consts)
            seq = [(l0, ib) for ib in range(L0["NBLK"])] + \
                  [(l1, ib) for ib in range(L1["NBLK"])]
            l0.setup()
            prev = None
            for lr, ib in seq:
                if lr is l1 and ib == 0:
                    l1.setup()
                cur = (lr, lr.front_a(ib))
                if lr is l0 and ib == 0:
                    for t, d in ((consts["hr"], hr_d), (consts["fold"], fold_d),
                                 (consts["den"], den_d), (consts["exp"], exp_d),
                                 (consts["halo"], halo_d)):
                        nc.sync.dma_start(out=t[:], in_=d[:])
                if prev is not None:
                    prev[0].back1(prev[1])
                lr.front_b(cur[1])
                if prev is not None:
                    prev[0].back2a(prev[1])
                lr.front_c(cur[1])
                if prev is not None:
                    prev[0].back2b(prev[1])
                prev = cur
            prev[0].back1(prev[1])
            prev[0].back2a(prev[1])
            prev[0].back2b(prev[1])
    nc.finalize()
    return nc


def _make_masks():
    import ml_dtypes
    hr = np.zeros((128, 2, K2, 72), np.float32)
    fold = np.zeros((72, 2, K2, 128), np.float32)
    den = np.zeros((72, 8), np.float32)
    expm = np.zeros((8, 72), np.float32)
    sc = 1.0 / math.sqrt(HD)
    for p in range(128):
        for ki in range(2):
            h = (ki * 128 + p) // HD
            for k2 in range(K2):
                hr[p, ki, k2, k2 * 8 + h] = sc
                fold[k2 * 8 + h, ki, k2, p] = 1.0
    for k2 in range(K2):
        for h in range(8):
            den[k2 * 8 + h, h] = 1.0
            expm[h, k2 * 8 + h] = 1.0
    bf = ml_dtypes.bfloat16
    return hr.astype(bf), fold.astype(bf), den, expm


def _shard_rows(x, lo, hi):
    """rows [lo, hi) of x[:, R, :] with zero padding outside."""
    Cc, R, Xc = x.shape
    out = np.zeros((Cc, hi - lo, Xc), x.dtype)
    a, b = max(lo, 0), min(hi, R)
    out[:, a - lo : b - lo] = x[:, a:b]
    return out


def kernel(p3, p4, p5, Wq0, Wk0, Wv0, Ww0_0, Ww0_1, Wq1, Wk1, Wv1, Ww1_0, Ww1_1):
    global _PROGRAM, _last_in_maps
    import ml_dtypes
    from concourse.bass_utils import run_bass_kernel_spmd

    bf = ml_dtypes.bfloat16
    if _PROGRAM is None:
        _PROGRAM = build_program()
    nc = _PROGRAM

    p3 = np.asarray(p3); p4 = np.asarray(p4); p5 = np.asarray(p5)
    hr, fold, den, expm = _make_masks()
    const_map = dict(hr_mask=hr, fold_mask=fold, den_mask=den, exp_mask=expm)
    for nm, w in (("wqT0", Wq0), ("wkT0", Wk0), ("wvT0", Wv0),
                  ("wqT1", Wq1), ("wkT1", Wk1), ("wvT1", Wv1)):
        const_map[nm] = np.ascontiguousarray(np.asarray(w).T).astype(bf)
    for nm, w in (("wo0T0", Ww0_0), ("wo1T0", Ww0_1),
                  ("wo0T1", Ww1_0), ("wo1T1", Ww1_1)):
        const_map[nm] = np.ascontiguousarray(np.asarray(w).T.astype(np.float32))

    in_maps = []
    for b in range(4):
        for half in range(2):
            t0_0 = 0 if half == 0 else 32   # layer0 top start
            t0_1 = 0 if half == 0 else 16   # layer1 top start
            m = dict(const_map)
            m["bot0"] = _shard_rows(p3[b], 2 * t0_0 - 1, 2 * t0_0 + L0["U"] - 1).astype(bf)
            m["top0"] = _shard_rows(p4[b], t0_0, t0_0 + L0["T"]).astype(bf)
            m["bot1"] = _shard_rows(p4[b], 2 * t0_1 - 1, 2 * t0_1 + L1["U"] - 1).astype(bf)
            m["top1"] = _shard_rows(p5[b], t0_1, t0_1 + L1["T"]).astype(bf)
            m["halo"] = np.full((72, 1), 1.0 if half == 0 else 0.0, np.float32)
            in_maps.append(m)

    _last_in_maps = in_maps
    res = run_bass_kernel_spmd(nc, in_maps, list(range(8))).results

    r3 = np.empty((4, C, 128, 128), np.float32)
    r4 = np.empty((4, C, 64, 64), np.float32)
    for b in range(4):
        r3[b, :, 0:64] = res[2 * b]["out0"]
        r3[b, :, 64:128] = res[2 * b + 1]["out0"]
        r4[b, :, 0:32] = res[2 * b]["out1"]
        r4[b, :, 32:64] = res[2 * b + 1]["out1"]
    return (r3, r4, np.asarray(p5, np.float32))


# revision 39
# speedup vs baseline: 1.0019x; 1.0019x over previous
"""Trainium2 Bass kernel for nn_AttnTopDown (sparse local attention, 2 layers).

Sharding: 8 cores = 4 batches x 2 spatial halves (top-grid row halves, 1 halo
row).  Each core computes both attn layers on its shard; p5 passes through on
host.

Per-core algorithm (channel-major: SBUF partition p holds channels p and
p+128):
  q = Wq@bot, v = Wv@bot, k = Wk@top       (PE)
  prod_k2 = shift(q, k2) * k               (DVE, 9 window offsets)
  logits72[(k2,h), pos] = masked-matmul(prod)   (PE, M=72 accumulate)
  exp -> den (PE mask reduce) -> recip -> r72 (PE expand) -> wnorm
  S_exp[c, botpos] = sum_k2 fold-matmuls of wnorm with parity-shifted views
  out = v . S_exp ; y = Ww1 @ relu(Ww0 @ out)    (DVE + PE)

Blocks of RT top rows are software-pipelined: softmax(i-1), convs(i),
fold/y-convs(i-1) so every engine keeps dense independent work.
"""

import math
import numpy as np

C = 256
NH = 8
HD = 32  # head dim
K2 = 9

# layer geometry
L0 = dict(T=33, W=64, X=128, U=67, RT=8, NBLK=4)
L1 = dict(T=17, W=32, X=64, U=35, RT=8, NBLK=2)

_PROGRAM = None
_last_in_maps = None


class _LayerRun:
    def __init__(self, nc, tc, pools, L, io, consts):
        import concourse.mybir as mybir
        self.nc = nc
        self.mybir = mybir
        self.L, self.io, self.consts = L, io, consts
        (self.wpool, self.kpool, self.inpool, self.qpool, self.vpool,
         self.prodpool, self.attpool, self.outpool, self.ypool,
         self.mmps) = pools
        self.T, self.W, self.X, self.U = L["T"], L["W"], L["X"], L["U"]
        self.RT, self.NBLK = L["RT"], L["NBLK"]
        self.BR = 2 * self.RT + 3
        self.PW = self.X + 2
        self.TWP = self.W + 1
        self.qr = 512 // self.X
        self.RC = 512 // self.W

    def cview(self, d):
        return d[:].rearrange("(ki p) r x -> p ki r x", p=128)

    def setup(self):
        nc, dt = self.nc, self.mybir.dt
        self.wt = {}
        for nm, key in (("k", "wkT"), ("q", "wqT"), ("v", "wvT"),
                        ("o0", "wo0T"), ("o1", "wo1T")):
            d = self.io[key]
            t = self.wpool.tile([128, 2, 2, 128], d.dtype, tag=f"w{nm}", name=f"w{nm}")
            nc.sync.dma_start(out=t[:], in_=d[:].rearrange(
                "(ki p) (mi m) -> p ki mi m", p=128, m=128))
            self.wt[nm] = t
            if nm == "k":
                T, W = self.T, self.W
                ts = min(2 * (512 // W), (T // 2) // (512 // W) * (512 // W))
                ts = max(ts, 512 // W)
                ka = self.kpool.tile([128, 2, ts, W], self.io["top"].dtype,
                                     tag="ktile_a", name="ktile_a")
                nc.sync.dma_start(out=ka[:], in_=self.cview(self.io["top"])[:, :, :ts, :])
                kb = self.kpool.tile([128, 2, T - ts, W], self.io["top"].dtype,
                                     tag="ktile_b", name="ktile_b")
                nc.sync.dma_start(out=kb[:], in_=self.cview(self.io["top"])[:, :, ts:, :])

        T, W = self.T, self.W
        ts = min(2 * (512 // W), (T // 2) // (512 // W) * (512 // W))
        ts = max(ts, 512 // W)
        self.k_sb = self.kpool.tile([128, 2, T, W], dt.bfloat16, tag="k_sb", name="k_sb")
        kr = 512 // W
        r0 = 0
        while r0 < T:
            nr = min(kr, T - r0)
            src = ka if r0 + nr <= ts else kb
            off = 0 if r0 + nr <= ts else ts
            for mi in range(2):
                ps = self.mmps.tile([128, 512], dt.float32, tag="mmps", name="mmps")
                for ki in range(2):
                    nc.tensor.matmul(
                        ps[:, : nr * W],
                        self.wt["k"][:, ki, mi, :],
                        src[:, ki, r0 - off : r0 - off + nr, :],
                        start=(ki == 0), stop=(ki == 1))
                nc.scalar.copy(out=self.k_sb[:, mi, r0 : r0 + nr, :],
                               in_=ps[:, : nr * W])
            r0 += nr

    def front_a(self, ib):
        nc, dt = self.nc, self.mybir.dt
        RT, W, X, BR, PW, qr = self.RT, self.W, self.X, self.BR, self.PW, self.qr
        ta = ib * RT
        bt = self.inpool.tile([128, 2, BR, X], self.io["bot"].dtype, tag="bt", name="bt")
        nc.sync.dma_start(out=bt[:],
                          in_=self.cview(self.io["bot"])[:, :, 2 * ta : 2 * ta + BR, :])

        qpad = self.qpool.tile([128, 2, BR, PW], dt.bfloat16, tag="qpad", name="qpad")
        nc.vector.memset(qpad[:, :, :, 0:1], 0.0)
        nc.vector.memset(qpad[:, :, :, X + 1 : X + 2], 0.0)
        st = dict(ib=ib, ta=ta, bt=bt, qpad=qpad)
        self._qconv(st, 0)
        self._qconv(st, self.qr)
        self._qconv(st, 2 * self.qr)
        return st

    def _qconv(self, st, r0):
        nc, dt = self.nc, self.mybir.dt
        X, BR, qr = self.X, self.BR, self.qr
        nr = min(qr, BR - r0)
        for mi in range(2):
            ps = self.mmps.tile([128, 512], dt.float32, tag="mmps", name="mmps")
            for ki in range(2):
                nc.tensor.matmul(
                    ps[:, : nr * X],
                    self.wt["q"][:, ki, mi, :],
                    st["bt"][:, ki, r0 : r0 + nr, :],
                    start=(ki == 0), stop=(ki == 1))
            nc.scalar.copy(
                out=st["qpad"][:, mi, r0 : r0 + nr, 1 : X + 1], in_=ps[:, : nr * X])

    def front_b(self, st):
        nc, dt = self.nc, self.mybir.dt
        RT, W, X, BR, qr = self.RT, self.W, self.X, self.BR, self.qr
        r0 = 3 * qr
        while r0 < BR:
            self._qconv(st, r0)
            r0 += min(qr, BR - r0)

        vtile = self.vpool.tile([128, 2, 2 * RT, X], dt.bfloat16, tag="vtile", name="vtile")
        for c in range(2 * RT // qr):
            r0 = c * qr
            for mi in range(2):
                ps = self.mmps.tile([128, 512], dt.float32, tag="mmps", name="mmps")
                for ki in range(2):
                    nc.tensor.matmul(
                        ps[:, : qr * X],
                        self.wt["v"][:, ki, mi, :],
                        st["bt"][:, ki, 1 + r0 : 1 + r0 + qr, :],
                        start=(ki == 0), stop=(ki == 1))
                nc.scalar.copy(out=vtile[:, mi, r0 : r0 + qr, :], in_=ps[:, : qr * X])
        st["vtile"] = vtile

    def front_c(self, st):
        nc, dt = self.nc, self.mybir.dt
        RT, W = self.RT, self.W
        ta, qpad = st["ta"], st["qpad"]
        prods = []
        for k2 in range(K2):
            di, dj = k2 // 3, k2 % 3
            pr = self.prodpool.tile([128, 2, RT + 1, W], dt.bfloat16,
                                    tag="prod", name="prod")
            qv = qpad[:, :, di : di + 2 * RT + 1 : 2, dj : dj + 2 * W : 2]
            kv = self.k_sb[:, :, ta : ta + RT + 1, :]
            nc.vector.tensor_mul(pr[:], qv, kv)
            prods.append(pr)

        hr_sb = self.consts["hr"]
        TWB = (RT + 1) * W
        lg = self.mmps.tile([72, TWB], dt.float32, tag="lgw", name="lgw", bufs=1)
        rr = 0
        while rr < RT + 1:
            nr = min(self.RC, RT + 1 - rr)
            nn = nr * W
            for k2 in range(K2):
                for ki in range(2):
                    nc.tensor.matmul(
                        lg[:, rr * W : rr * W + nn],
                        hr_sb[:, ki, k2, :],
                        prods[k2][:, ki, rr : rr + nr, :],
                        start=(k2 == 0 and ki == 0),
                        stop=(k2 == K2 - 1 and ki == 1))
            rr += nr
        st["lg"] = lg

    def back1(self, st):
        nc, dt = self.nc, self.mybir.dt
        AF = self.mybir.ActivationFunctionType
        RT, W, TWP = self.RT, self.W, self.TWP
        den_sb, exp_sb = self.consts["den"], self.consts["exp"]
        TWB = (RT + 1) * W
        lg = st["lg"]
        w72 = self.attpool.tile([72, RT + 1, TWP], dt.bfloat16, tag="w72", name="w72")
        nc.vector.memset(w72[:, :, W : W + 1], 0.0)
        e72 = self.attpool.tile([72, TWB], dt.float32r, tag="e72", name="e72")
        nc.scalar.activation(out=e72[:], in_=lg[:], func=AF.Exp)
        rec = self.attpool.tile([8, TWB], dt.float32r, tag="rec", name="rec")
        for n0 in range(0, TWB, 512):
            nn = min(512, TWB - n0)
            den = self.mmps.tile([8, 512], dt.float32, tag="mmps", name="den")
            nc.tensor.matmul(den[:, :nn], den_sb[:], e72[:, n0 : n0 + nn],
                             start=True, stop=True)
            nc.vector.reciprocal(out=rec[:, n0 : n0 + nn], in_=den[:, :nn])
        r72 = self.mmps.tile([72, TWB], dt.float32, tag="lgw", name="r72", bufs=1)
        for n0 in range(0, TWB, 512):
            nn = min(512, TWB - n0)
            nc.tensor.matmul(r72[:, n0 : n0 + nn], exp_sb[:], rec[:, n0 : n0 + nn],
                             start=True, stop=True)
        nc.vector.tensor_mul(
            w72[:, :, 0:W],
            e72[:].rearrange("p (a b) -> p a b", b=W),
            r72[:].rearrange("p (a b) -> p a b", b=W))
        if st["ib"] == self.NBLK - 1:
            nc.vector.tensor_scalar_mul(w72[:, RT, 0:W], w72[:, RT, 0:W],
                                        self.consts["halo"][:])
        st["w72"] = w72

    def back2a(self, st):
        nc, dt = self.nc, self.mybir.dt
        RT, W, X = self.RT, self.W, self.X
        fold_sb = self.consts["fold"]
        vtile, w72 = st["vtile"], st["w72"]
        outb = self.outpool.tile([128, 2, 2 * RT, X], dt.float32r, tag="outb", name="outb")
        for ki in range(2):
            for pu in range(2):
                for pv in range(2):
                    terms = [(di, dj)
                             for di in ((1,) if pu == 0 else (0, 2))
                             for dj in ((1,) if pv == 0 else (0, 2))]
                    nn = RT * (X // 2)
                    se = self.mmps.tile([128, 512], dt.float32, tag="mmps", name="se")
                    for i, (di, dj) in enumerate(terms):
                        dr, dc = (1 if di == 0 else 0), (1 if dj == 0 else 0)
                        k2 = di * 3 + dj
                        wv_ = w72[:, dr : dr + RT, dc : dc + W]
                        nc.tensor.matmul(
                            se[:, :nn], fold_sb[:, ki, k2, :], wv_,
                            start=(i == 0), stop=(i == len(terms) - 1))
                    nc.vector.tensor_mul(
                        outb[:, ki, pu :: 2, pv :: 2],
                        vtile[:, ki, pu :: 2, pv :: 2],
                        se[:, :nn].rearrange("p (a b) -> p a b", b=W))
        st["outb"] = outb

    def back2b(self, st):
        nc, dt = self.nc, self.mybir.dt
        AF = self.mybir.ActivationFunctionType
        RT, W, X, qr = self.RT, self.W, self.X, self.qr
        ta, outb = st["ta"], st["outb"]
        for c in range(2 * RT // qr):
            r0 = c * qr
            y0r = self.ypool.tile([128, 2, qr * X], dt.float32r, tag="y0r", name="y0r")
            for mi in range(2):
                ps = self.mmps.tile([128, 512], dt.float32, tag="mmps", name="mmps")
                for ki in range(2):
                    nc.tensor.matmul(
                        ps[:, : qr * X],
                        self.wt["o0"][:, ki, mi, :],
                        outb[:, ki, r0 : r0 + qr, :],
                        start=(ki == 0), stop=(ki == 1))
                nc.scalar.activation(out=y0r[:, mi, :], in_=ps[:, : qr * X],
                                     func=AF.Relu)
            yout = self.ypool.tile([128, 2, qr, X], dt.float32, tag="yout", name="yout")
            for mi in range(2):
                ps = self.mmps.tile([128, 512], dt.float32, tag="mmps", name="mmps")
                for ki in range(2):
                    nc.tensor.matmul(
                        ps[:, : qr * X],
                        self.wt["o1"][:, ki, mi, :],
                        y0r[:, ki, :],
                        start=(ki == 0), stop=(ki == 1))
                if mi == 0:
                    nc.vector.tensor_copy(yout[:, mi, :, :], ps[:, : qr * X])
                else:
                    nc.scalar.copy(out=yout[:, mi, :, :], in_=ps[:, : qr * X])
            nc.sync.dma_start(
                out=self.cview(self.io["out"])[:, :, 2 * ta + r0 : 2 * ta + r0 + qr, :],
                in_=yout[:])


def build_program():
    import concourse.bacc as bacc
    import concourse.mybir as mybir
    import concourse.tile as tile
    from contextlib import ExitStack

    dt = mybir.dt
    nc = bacc.Bacc(None, target_bir_lowering=False)

    def make_io(sfx, L):
        return dict(
            bot=nc.dram_tensor(f"bot{sfx}", [C, L["U"], L["X"]], dt.bfloat16,
                               kind="ExternalInput"),
            top=nc.dram_tensor(f"top{sfx}", [C, L["T"], L["W"]], dt.bfloat16,
                               kind="ExternalInput"),
            out=nc.dram_tensor(f"out{sfx}", [C, 2 * L["RT"] * L["NBLK"], L["X"]],
                               dt.float32, kind="ExternalOutput"),
            wqT=nc.dram_tensor(f"wqT{sfx}", [C, C], dt.bfloat16, kind="ExternalInput"),
            wkT=nc.dram_tensor(f"wkT{sfx}", [C, C], dt.bfloat16, kind="ExternalInput"),
            wvT=nc.dram_tensor(f"wvT{sfx}", [C, C], dt.bfloat16, kind="ExternalInput"),
            wo0T=nc.dram_tensor(f"wo0T{sfx}", [C, C], dt.float32r, kind="ExternalInput"),
            wo1T=nc.dram_tensor(f"wo1T{sfx}", [C, C], dt.float32r, kind="ExternalInput"),
        )

    io0, io1 = make_io(0, L0), make_io(1, L1)
    hr_d = nc.dram_tensor("hr_mask", [128, 2, K2, 72], dt.bfloat16, kind="ExternalInput")
    fold_d = nc.dram_tensor("fold_mask", [72, 2, K2, 128], dt.bfloat16, kind="ExternalInput")
    den_d = nc.dram_tensor("den_mask", [72, 8], dt.float32r, kind="ExternalInput")
    exp_d = nc.dram_tensor("exp_mask", [8, 72], dt.float32r, kind="ExternalInput")
    halo_d = nc.dram_tensor("halo", [72, 1], dt.float32, kind="ExternalInput")

    with tile.TileContext(nc) as tc:
        octx0 = ExitStack()
        octx0.enter_context(nc.allow_low_precision(reason="f32r rounding is intentional"))
        with octx0, ExitStack() as octx:
            cpool = octx.enter_context(tc.tile_pool(name="consts", bufs=1))
            consts = dict(
                hr=cpool.tile([128, 2, K2, 72], dt.bfloat16, tag="hr", name="hr"),
                fold=cpool.tile([72, 2, K2, 128], dt.bfloat16, tag="fold", name="fold"),
                den=cpool.tile([72, 8], dt.float32r, tag="den", name="den"),
                exp=cpool.tile([8, 72], dt.float32r, tag="exp", name="exp"),
                halo=cpool.tile([72, 1], dt.float32, tag="halo", name="halo"),
            )
            pools = (
                octx.enter_context(tc.tile_pool(name="wpool", bufs=1)),
                octx.enter_context(tc.tile_pool(name="kpool", bufs=1)),
                octx.enter_context(tc.tile_pool(name="inpool", bufs=2)),
                octx.enter_context(tc.tile_pool(name="qpool", bufs=2)),
                octx.enter_context(tc.tile_pool(name="vpool", bufs=2)),
                octx.enter_context(tc.tile_pool(name="prodpool", bufs=12)),
                octx.enter_context(tc.tile_pool(name="attpool", bufs=4)),
                octx.enter_context(tc.tile_pool(name="outpool", bufs=2)),
                octx.enter_context(tc.tile_pool(name="ypool", bufs=3)),
                octx.enter_context(tc.tile_pool(name="mmps", bufs=6, space="PSUM")),
            )
            # all blocks of both layers in one software pipeline
            l0 = _LayerRun(nc, tc, pools, L0, io0, consts)
            l1 = _LayerRun(nc, tc, pools, L1, io1, consts)
            seq = [(l0, ib) for ib in range(L0["NBLK"])] + \
                  [(l1, ib) for ib in range(L1["NBLK"])]
            l0.setup()
            prev = None
            for lr, ib in seq:
                if lr is l1 and ib == 0:
                    l1.setup()
                cur = (lr, lr.front_a(ib))
                if lr is l0 and ib == 0:
                    for t, d in ((consts["hr"], hr_d), (consts["fold"], fold_d),
                                 (consts["den"], den_d), (consts["exp"], exp_d),
                                 (consts["halo"], halo_d)):
                        nc.sync.dma_start(out=t[:], in_=d[:])
                if prev is not None:
                    prev[0].back1(prev[1])
                lr.front_b(cur[1])
                if prev is not None:
                    prev[0].back2a(prev[1])
                lr.front_c(cur[1])
                if prev is not None:
                    prev[0].back2b(prev[1])
                prev = cur
            prev[0].back1(prev[1])
            prev[0].back2a(prev[1])
            prev[0].back2b(prev[1])
    nc.finalize()
    return nc


def _make_masks():
    import ml_dtypes
    hr = np.zeros((128, 2, K2, 72), np.float32)
    fold = np.zeros((72, 2, K2, 128), np.float32)
    den = np.zeros((72, 8), np.float32)
    expm = np.zeros((8, 72), np.float32)
    sc = 1.0 / math.sqrt(HD)
    for p in range(128):
        for ki in range(2):
            h = (ki * 128 + p) // HD
            for k2 in range(K2):
                hr[p, ki, k2, k2 * 8 + h] = sc
                fold[k2 * 8 + h, ki, k2, p] = 1.0
    for k2 in range(K2):
        for h in range(8):
            den[k2 * 8 + h, h] = 1.0
            expm[h, k2 * 8 + h] = 1.0
    bf = ml_dtypes.bfloat16
    return hr.astype(bf), fold.astype(bf), den, expm


def _shard_rows(x, lo, hi):
    """rows [lo, hi) of x[:, R, :] with zero padding outside."""
    Cc, R, Xc = x.shape
    out = np.zeros((Cc, hi - lo, Xc), x.dtype)
    a, b = max(lo, 0), min(hi, R)
    out[:, a - lo : b - lo] = x[:, a:b]
    return out


def kernel(p3, p4, p5, Wq0, Wk0, Wv0, Ww0_0, Ww0_1, Wq1, Wk1, Wv1, Ww1_0, Ww1_1):
    global _PROGRAM, _last_in_maps
    import ml_dtypes
    from concourse.bass_utils import run_bass_kernel_spmd

    bf = ml_dtypes.bfloat16
    if _PROGRAM is None:
        _PROGRAM = build_program()
    nc = _PROGRAM

    p3 = np.asarray(p3); p4 = np.asarray(p4); p5 = np.asarray(p5)
    hr, fold, den, expm = _make_masks()
    const_map = dict(hr_mask=hr, fold_mask=fold, den_mask=den, exp_mask=expm)
    for nm, w in (("wqT0", Wq0), ("wkT0", Wk0), ("wvT0", Wv0),
                  ("wqT1", Wq1), ("wkT1", Wk1), ("wvT1", Wv1)):
        const_map[nm] = np.ascontiguousarray(np.asarray(w).T).astype(bf)
    for nm, w in (("wo0T0", Ww0_0), ("wo1T0", Ww0_1),
                  ("wo0T1", Ww1_0), ("wo1T1", Ww1_1)):
        const_map[nm] = np.ascontiguousarray(np.asarray(w).T.astype(np.float32))

    in_maps = []
    for b in range(4):
        for half in range(2):
            t0_0 = 0 if half == 0 else 32   # layer0 top start
            t0_1 = 0 if half == 0 else 16   # layer1 top start
            m = dict(const_map)
            m["bot0"] = _shard_rows(p3[b], 2 * t0_0 - 1, 2 * t0_0 + L0["U"] - 1).astype(bf)
            m["top0"] = _shard_rows(p4[b], t0_0, t0_0 + L0["T"]).astype(bf)
            m["bot1"] = _shard_rows(p4[b], 2 * t0_1 - 1, 2 * t0_1 + L1["U"] - 1).astype(bf)
            m["top1"] = _shard_rows(p5[b], t0_1, t0_1 + L1["T"]).astype(bf)
            m["halo"] = np.full((72, 1), 1.0 if half == 0 else 0.0, np.float32)
            in_maps.append(m)

    _last_in_maps = in_maps
    res = run_bass_kernel_spmd(nc, in_maps, list(range(8))).results

    r3 = np.empty((4, C, 128, 128), np.float32)
    r4 = np.empty((4, C, 64, 64), np.float32)
    for b in range(4):
        r3[b, :, 0:64] = res[2 * b]["out0"]
        r3[b, :, 64:128] = res[2 * b + 1]["out0"]
        r4[b, :, 0:32] = res[2 * b]["out1"]
        r4[b, :, 32:64] = res[2 * b + 1]["out1"]
    return (r3, r4, np.asarray(p5, np.float32))
